# revision 36
# baseline (speedup 1.0000x reference)
"""Causal GQA self-attention (B=2, S=2048, D=2048, 16 heads / 4 KV heads) on 8
Trainium2 NeuronCores.

Sharding: tensor-parallel over heads. Core c owns Q heads (2c, 2c+1) and KV
head c//2. Each core computes its heads' attention output and a partial
output projection (columns of Wp.T owned by its heads); the host sums the 8
partial outputs.

v2 design (vs baseline):
  - All matmuls in bf16 (PSUM accumulation stays f32). x, weights, and the
    partial outputs move over DMA in bf16, halving HBM traffic.
  - Attention scores are computed directly transposed: sc[key, query] =
    kT_tile^T @ qT_group, so the exp'd tiles feed the AV matmul with no PE
    transposes and no PSUM->SBUF copy pass. The softmax denominator comes
    from a ones-stationary matmul accumulated in PSUM [1, 512].
  - Causal masking: only the 4 in-group (diagonal) tiles per (g,h) need a
    triangular mask add; above-diagonal regions exp() to exactly 0 and
    contribute nothing to AV / the denominator.
  - cos/sin RoPE tables are SBUF-resident constants (identical across
    batches), not per-tile DMAs.
"""

import math

import numpy as np

B = 2
S = 2048
D = 2048
T = B * S
NH = 16
NKV = 4
HD = 128
P = 128
ROPE_BASE = 10000.0
EPS = float(np.finfo(np.float32).eps)
NEG = -1.0e30

N_CORES = 8
TT_B = S // P          # 16 token tiles per batch
GROUPS = 4             # groups of 4 q-tiles (512 queries)
QKV = 512              # per-core fused projection width: 2*q + k + v heads
HB = 8                 # token tiles per rsqrt batch

_PROG = {}


def _build_program(loop_n=0):
    import concourse.mybir as mybir
    import concourse.tile as tile
    from concourse import bacc
    from concourse.masks import make_identity

    f32 = mybir.dt.float32
    bf16 = mybir.dt.bfloat16
    AL = mybir.AluOpType
    AF = mybir.ActivationFunctionType
    AX = mybir.AxisListType

    nc = bacc.Bacc("TRN2", target_bir_lowering=False, debug=False,
                   enable_asserts=True, num_devices=N_CORES)

    xT = nc.dram_tensor("xT", [D, T], bf16, kind="ExternalInput").ap()
    wcat = nc.dram_tensor("wcat", [D, QKV], bf16, kind="ExternalInput").ap()
    wp = nc.dram_tensor("wp", [2 * HD, D], bf16, kind="ExternalInput").ap()
    cosd = nc.dram_tensor("cosd", [S, HD // 2], bf16, kind="ExternalInput").ap()
    sind = nc.dram_tensor("sind", [S, HD // 2], bf16, kind="ExternalInput").ap()
    rmaskd = nc.dram_tensor("rmaskd", [P, P], f32, kind="ExternalInput").ap()
    gaind = nc.dram_tensor("gaind", [P, 4], f32, kind="ExternalInput").ap()
    outd = nc.dram_tensor("out", [T, D], bf16, kind="ExternalOutput").ap()

    xT_r = xT.rearrange("(kt p) t -> p kt t", p=P)        # [128, 16, T]
    wcat_r = wcat.rearrange("(kt p) n -> p kt n", p=P)    # [128, 16, 512]
    wp_r = wp.rearrange("(ct p) o -> p ct o", p=P)        # [128, 2, D]
    cos_r = cosd.rearrange("(t p) c -> p t c", p=P)       # [128, 16, 64]
    sin_r = sind.rearrange("(t p) c -> p t c", p=P)

    import contextlib as _ctxlib
    with tile.TileContext(nc) as tc, _ctxlib.ExitStack() as _es:
        pc = _es.enter_context(tc.tile_pool(name="const", bufs=1))
        pb = _es.enter_context(tc.tile_pool(name="batch", bufs=1))
        px = _es.enter_context(tc.tile_pool(name="xs", bufs=2))
        pw = _es.enter_context(tc.tile_pool(name="work", bufs=2))
        pat = _es.enter_context(tc.tile_pool(name="attn", bufs=2))
        psm = _es.enter_context(tc.tile_pool(name="small", bufs=4))
        po = _es.enter_context(tc.tile_pool(name="outp", bufs=3))
        prl = _es.enter_context(tc.tile_pool(name="rlp", bufs=2))
        prq = _es.enter_context(tc.tile_pool(name="rlq", bufs=2))
        pyt = _es.enter_context(tc.tile_pool(name="ytp", bufs=2))
        # PSUM budget (8 banks): ppA serves tags pmm (qkv accum + out-proj,
        # phase-disjoint), sc, ya at 2 bufs each = 6 banks; ppB serves ptt
        # (bf16 transpose staging) + lb ([1,512] row) at 1 buf each = 2.
        ppA = _es.enter_context(tc.tile_pool(name="psA", bufs=2, space="PSUM"))
        ppB = _es.enter_context(tc.tile_pool(name="psB", bufs=1, space="PSUM"))

        # ---- constants resident in SBUF
        wcat_sb = pc.tile([P, TT_B, QKV], bf16, tag="wcat")
        for kt in range(TT_B):
            nc.sync.dma_start(wcat_sb[:, kt, :], wcat_r[:, kt, :])
        wp_sb = pc.tile([P, 2, D], bf16, tag="wp")
        nc.sync.dma_start(wp_sb[:], wp_r[:])
        cos_sb = pc.tile([P, TT_B, HD // 2], bf16, tag="cos")
        nc.sync.dma_start(cos_sb[:], cos_r[:])
        sin_sb = pc.tile([P, TT_B, HD // 2], bf16, tag="sin")
        nc.sync.dma_start(sin_sb[:], sin_r[:])
        rmask_sb = pc.tile([P, P], f32, tag="rmask")
        nc.sync.dma_start(rmask_sb[:], rmaskd[:])
        gain_sb = pc.tile([P, 4], f32, tag="gain")
        nc.sync.dma_start(gain_sb[:], gaind[:])
        idf = pc.tile([P, P], f32, tag="idf")
        make_identity(nc, idf[:])
        idb = pc.tile([P, P], bf16, tag="idb")
        nc.vector.tensor_copy(idb[:], idf[:])
        ones_sb = pc.tile([P, 1], bf16, tag="ones")
        nc.vector.memset(ones_sb[:], 1.0)

        prev_tail = None
        for b in [bb % B for bb in range(B * max(1, loop_n))]:
            # qkT packs [q0, q1, k] transposed heads: [128, 3, 1024] per half
            qkT_h = [pb.tile([P, 3, S // 2], bf16, tag="qkTlo", name="qkTlo"),
                     pb.tile([P, 3, S // 2], bf16, tag="qkThi", name="qkThi")]
            vN_h = [pb.tile([P, TT_B // 2, HD], bf16, tag="vNlo", name="vNlo"),
                    pb.tile([P, TT_B // 2, HD], bf16, tag="vNhi", name="vNhi")]

            def qT_grp(hh, g):
                half_i, loc = divmod(g * 4, TT_B // 2)
                return qkT_h[half_i][:, hh, loc * P:(loc + 4) * P]

            def kT_at(jt):
                half_i, loc = divmod(jt, TT_B // 2)
                return qkT_h[half_i][:, 2, loc * P:(loc + 1) * P]

            def vN_at(jt):
                half_i, loc = divmod(jt, TT_B // 2)
                return vN_h[half_i][:, loc, :]

            yT = pyt.tile([P, 2, S], bf16, tag="yT")

            # ======= QKV projection + RMS + RoPE (software-pipelined) =====
            # All projection matmuls are emitted first so the PE queue never
            # waits on the DVE/Pool rope chain; rope+transpose quarters are
            # interleaved between attention groups below.
            stgs = {}
            xts = {}
            ssqs = {}
            rsgs = {}

            def project_quarter(qq):
                hh = qq // 2
                if qq % 2 == 0:
                    ssqs[hh] = pb.tile([P, HB, 3], f32, tag=f"ssq{hh}",
                                       name=f"ssq{hh}")
                for tt in range(4 * qq, 4 * qq + 4):
                    t0 = b * S + tt * P
                    if tt % 4 == 0:
                        xt = px.tile([P, TT_B, 4 * P], bf16, tag="xt")
                        xts[qq] = xt
                        nc.sync.dma_start(xt[:], xT_r[:, :, t0:t0 + 4 * P])
                    xt = xts[qq]
                    xoff = (tt % 4) * P

                    pp = ppA.tile([P, QKV], f32, tag="pmm", name="pmm")
                    for kt in range(TT_B):
                        nc.tensor.matmul(pp[:], xt[:, kt, xoff:xoff + P],
                                         wcat_sb[:, kt, :],
                                         start=(kt == 0), stop=(kt == TT_B - 1))

                    # v: rounding copy straight out of PSUM
                    nc.scalar.copy(vN_at(tt), pp[:, 3 * HD:4 * HD])
                    # stage q0,q1,k in SBUF; sum-of-squares per segment
                    stg = pb.tile([P, 3 * HD], f32, tag=f"stg{tt}")
                    stgs[tt] = stg
                    nc.scalar.copy(stg[:], pp[:, :3 * HD])
                    scr = prq.tile([P, 3 * HD], f32, tag="scr")
                    nc.vector.tensor_tensor(scr[:], stg[:], stg[:], AL.mult)
                    nc.vector.tensor_reduce(
                        ssqs[hh][:, tt % HB, :],
                        scr[:].rearrange("p (s x) -> p s x", s=3),
                        axis=AX.X, op=AL.add)

            def rsqrt_half(hh):
                # rs = exp(-0.5*ln(ssq/HD+eps)) * gain
                lnb = pb.tile([P, HB, 3], f32, tag=f"lnb{hh}")
                nc.scalar.activation(lnb[:], ssqs[hh][:], AF.Ln,
                                     scale=1.0 / HD, bias=gain_sb[:, 3:4])
                rsb = pb.tile([P, HB, 3], f32, tag=f"rsb{hh}")
                nc.scalar.activation(rsb[:], lnb[:], AF.Exp, scale=-0.5)
                rsg = pb.tile([P, HB, 3], f32, tag=f"rsg{hh}")
                nc.vector.tensor_tensor(
                    rsg[:], rsb[:],
                    gain_sb[:, None, :3].to_broadcast([P, HB, 3]), AL.mult)
                rsgs[hh] = rsg

            def rope_quarter(qq):
                rsg = rsgs[qq // 2]
                for tt in range(4 * qq, 4 * qq + 4):
                    ppv = stgs[tt][:].rearrange("p (s x) -> p s x", s=3)
                    qn = pw.tile([P, 3, HD], bf16, tag="qn")
                    nc.vector.tensor_tensor(
                        qn[:], ppv,
                        rsg[:, tt % HB, :, None].to_broadcast([P, 3, HD]),
                        AL.mult)

                    # rope: out1 = a*cos + b2*sin ; out2 = b2*cos - a*sin
                    a = qn[:, :, :HD // 2]
                    b2 = qn[:, :, HD // 2:]
                    rpb = pw.tile([P, 3, HD], bf16, tag="rpb")
                    o1 = rpb[:, :, :HD // 2]
                    o2 = rpb[:, :, HD // 2:]
                    t1 = pw.tile([P, 3, HD // 2], bf16, tag="t1")
                    t2 = pw.tile([P, 3, HD // 2], bf16, tag="t2")
                    cb = cos_sb[:, tt:tt + 1, :].to_broadcast([P, 3, HD // 2])
                    sb_ = sin_sb[:, tt:tt + 1, :].to_broadcast([P, 3, HD // 2])
                    nc.gpsimd.tensor_tensor(t1[:], a, cb, AL.mult)
                    nc.vector.tensor_tensor(t2[:], b2, sb_, AL.mult)
                    nc.vector.tensor_tensor(o1, t1[:], t2[:], AL.add)
                    nc.gpsimd.tensor_tensor(t1[:], b2, cb, AL.mult)
                    nc.gpsimd.tensor_tensor(t2[:], a, sb_, AL.mult)
                    nc.vector.tensor_tensor(o2, t1[:], t2[:], AL.subtract)

                    # transpose q0,q1,k into [head_dim, token] (bf16, 1 cyc/row)
                    rpf = rpb[:].rearrange("p s x -> p (s x)")
                    ptq = ppB.tile([P, 3 * P], bf16, tag="ptt", name="ptt")
                    for sseg in range(3):
                        nc.tensor.transpose(ptq[:, sseg * P:(sseg + 1) * P],
                                            rpf[:, sseg * P:(sseg + 1) * P],
                                            idb[:])
                    half_i, loc = divmod(tt, TT_B // 2)
                    nc.scalar.copy(
                        qkT_h[half_i][:, :, loc * P:(loc + 1) * P],
                        ptq[:].rearrange("p (s x) -> p s x", s=3))

            # ================= attention (+ interleaved out-proj) =========
            def proj_block(yT_t, bb, tt_list):
                for tt in tt_list:
                    ob = po.tile([P, D], bf16, tag="ob")
                    for oc in range(4):
                        pout = ppA.tile([P, 512], f32, tag="pmm", name="pout")
                        for ct in range(2):
                            nc.tensor.matmul(
                                pout[:], yT_t[:, ct, tt * P:(tt + 1) * P],
                                wp_sb[:, ct, oc * 512:(oc + 1) * 512],
                                start=(ct == 0), stop=(ct == 1))
                        if oc % 2 == 0:
                            nc.vector.tensor_copy(ob[:, oc * 512:(oc + 1) * 512],
                                                  pout[:])
                        else:
                            nc.scalar.copy(ob[:, oc * 512:(oc + 1) * 512],
                                           pout[:])
                    nc.sync.dma_start(
                        outd[bb * S + tt * P: bb * S + (tt + 1) * P, :], ob[:])

            def attn_group(g):
                nj = 4 * (g + 1)       # key tiles for this group
                for h in range(2):
                    attnT = pat.tile([P, TT_B, 512], bf16, tag="attnT")
                    lb = ppB.tile([1, 512], f32, tag="lb", name="lb")
                    ya = ppA.tile([P, 512], f32, tag="ya", name="ya")
                    q_rhs = qT_grp(h, g)

                    def lav(jt, stop):
                        nc.tensor.matmul(lb[:], ones_sb[:],
                                         attnT[:, jt, :],
                                         start=(jt == 0), stop=stop)
                        nc.tensor.matmul(ya[:], vN_at(jt),
                                         attnT[:, jt, :],
                                         start=(jt == 0), stop=stop)

                    prev = None
                    for jt in range(nj):
                        sc = ppA.tile([P, 512], f32, tag="sc", name="sc")
                        jj = jt - 4 * g
                        c0 = jj * 128 if jj > 0 else 0
                        nc.tensor.matmul(sc[:], kT_at(jt), q_rhs,
                                         start=True, stop=True)
                        if jj >= 0:
                            # triangular mask on the boundary block
                            nc.vector.tensor_tensor(
                                sc[:, jj * 128:(jj + 1) * 128],
                                sc[:, jj * 128:(jj + 1) * 128],
                                rmask_sb[:], AL.add)
                        if c0 > 0:
                            nc.vector.memset(attnT[:, jt, :c0], 0.0)
                        nc.scalar.activation(attnT[:, jt, c0:], sc[:, c0:],
                                             AF.Exp)
                        if prev is not None:
                            lav(prev, stop=False)
                        prev = jt
                    lav(prev, stop=True)

                    rl = psm.tile([1, 512], f32, tag="rl")
                    nc.vector.reciprocal(rl[:], lb[:])
                    rlb = prl.tile([P, 512], f32, tag="rlb")
                    nc.gpsimd.partition_broadcast(rlb[:], rl[:])
                    nc.vector.tensor_tensor(
                        yT[:, h, g * 512:(g + 1) * 512], ya[:], rlb[:],
                        AL.mult)

                    # out-proj for the previous group's tokens: emitted here
                    # so its PE work lands after this group's first head and
                    # never waits on a fresh evac chain.
                    if h == 0 and g >= 1:
                        proj_block(yT, b, range(4 * (g - 1), 4 * g))

            project_quarter(0)
            project_quarter(1)
            rsqrt_half(0)
            project_quarter(2)
            rope_quarter(0)
            project_quarter(3)
            rope_quarter(1)
            rsqrt_half(1)
            attn_group(0)
            rope_quarter(2)
            attn_group(1)
            rope_quarter(3)
            attn_group(2)
            attn_group(3)
            proj_block(yT, b, range(4 * (GROUPS - 1), 4 * GROUPS))

    nc.compile()
    return nc


def _get_program(loop_n=0):
    key = loop_n
    if key not in _PROG:
        _PROG[key] = _build_program(loop_n)
    return _PROG[key]


def _host_prep(x, Wq, Wk, Wv, Wp, q_gain):
    """Build the 8 per-core input maps."""
    import ml_dtypes
    bf16 = ml_dtypes.bfloat16

    x = np.ascontiguousarray(x.reshape(T, D), dtype=np.float32)
    xT = np.ascontiguousarray(x.T.astype(bf16))          # [D, T] bf16

    inv_freq = 1.0 / (ROPE_BASE ** (np.arange(0, HD, 2, dtype=np.float32) / HD))
    freqs = np.arange(S, dtype=np.float32)[:, None] * inv_freq[None, :]
    cos = np.ascontiguousarray(np.cos(freqs).astype(bf16))   # [S, 64]
    sin = np.ascontiguousarray(np.sin(freqs).astype(bf16))

    r = np.arange(P)[:, None]
    c = np.arange(P)[None, :]
    rmask = np.where(c < r, NEG, 0.0).astype(np.float32)   # [128, 128] tri

    in_maps = []
    for core in range(N_CORES):
        h0 = 2 * core
        kv = core // 2
        WqT = Wq[h0 * HD:(h0 + 2) * HD, :].T             # [D, 256]
        WkT = Wk[kv * HD:(kv + 1) * HD, :].T             # [D, 128]
        WvT = Wv[kv * HD:(kv + 1) * HD, :].T             # [D, 128]
        wcat = np.ascontiguousarray(
            np.concatenate([WqT, WkT, WvT], axis=1).astype(bf16))
        wpT = np.ascontiguousarray(
            Wp[:, h0 * HD:(h0 + 2) * HD].T.astype(bf16))        # [256, D]
        scale = 1.0 / math.sqrt(HD)
        gain = np.tile(np.array(
            [[q_gain[h0] * scale, q_gain[h0 + 1] * scale, 1.0, EPS]],
            dtype=np.float32), (P, 1))
        in_maps.append({
            "xT": xT,
            "wcat": wcat,
            "wp": wpT,
            "cosd": cos,
            "sind": sin,
            "rmaskd": rmask,
            "gaind": np.ascontiguousarray(gain),
        })
    return in_maps


def kernel(x, Wq, Wk, Wv, Wp, q_gain):
    from concourse.bass_utils import run_bass_kernel_spmd

    nc = _get_program()
    in_maps = _host_prep(x, Wq, Wk, Wv, Wp, q_gain)
    try:
        res = run_bass_kernel_spmd(nc, in_maps, core_ids=list(range(N_CORES)))
    except Exception:
        # one retry: a previous crashed run can leave the exec unit wedged
        res = run_bass_kernel_spmd(nc, in_maps, core_ids=list(range(N_CORES)))
    total = np.zeros((T, D), dtype=np.float32)
    for r in res.results:
        total += r["out"].astype(np.float32)
    return total.reshape(B, S, D)


# revision 40
# speedup vs baseline: 1.2417x; 1.2417x over previous
"""Causal GQA self-attention (B=2, S=2048, D=2048, 16 heads / 4 KV heads) on 8
Trainium2 NeuronCores.

Sharding: tensor-parallel over heads. Core c owns Q heads (2c, 2c+1) and KV
head c//2. Each core computes its heads' attention output and a partial
output projection (columns of Wp.T owned by its heads); the host sums the 8
partial outputs.

v2 design (vs baseline):
  - All matmuls in bf16 (PSUM accumulation stays f32). x, weights, and the
    partial outputs move over DMA in bf16, halving HBM traffic.
  - Attention scores are computed directly transposed: sc[key, query] =
    kT_tile^T @ qT_group, so the exp'd tiles feed the AV matmul with no PE
    transposes and no PSUM->SBUF copy pass. The softmax denominator comes
    from a ones-stationary matmul accumulated in PSUM [1, 512].
  - Causal masking: only the 4 in-group (diagonal) tiles per (g,h) need a
    triangular mask add; above-diagonal regions exp() to exactly 0 and
    contribute nothing to AV / the denominator.
  - cos/sin RoPE tables are SBUF-resident constants (identical across
    batches), not per-tile DMAs.
"""

import math

import numpy as np

B = 2
S = 2048
D = 2048
T = B * S
NH = 16
NKV = 4
HD = 128
P = 128
ROPE_BASE = 10000.0
EPS = float(np.finfo(np.float32).eps)
NEG = -1.0e30

N_CORES = 8
TT_B = S // P          # 16 token tiles per batch
GROUPS = 4             # groups of 4 q-tiles (512 queries)
QKV = 512              # per-core fused projection width: 2*q + k + v heads
HB = 8                 # token tiles per rsqrt batch

_PROG = {}


def _build_program(loop_n=0):
    import concourse.mybir as mybir
    import concourse.tile as tile
    from concourse import bacc
    from concourse.masks import make_identity

    f32 = mybir.dt.float32
    bf16 = mybir.dt.bfloat16
    AL = mybir.AluOpType
    AF = mybir.ActivationFunctionType
    AX = mybir.AxisListType

    nc = bacc.Bacc("TRN2", target_bir_lowering=False, debug=False,
                   enable_asserts=True, num_devices=N_CORES)

    xT = nc.dram_tensor("xT", [D, T], bf16, kind="ExternalInput").ap()
    wcat = nc.dram_tensor("wcat", [D, QKV], bf16, kind="ExternalInput").ap()
    wp = nc.dram_tensor("wp", [2 * HD, D], bf16, kind="ExternalInput").ap()
    cosd = nc.dram_tensor("cosd", [S, HD // 2], bf16, kind="ExternalInput").ap()
    sind = nc.dram_tensor("sind", [S, HD // 2], bf16, kind="ExternalInput").ap()
    rmaskd = nc.dram_tensor("rmaskd", [P, P], f32, kind="ExternalInput").ap()
    gaind = nc.dram_tensor("gaind", [P, 4], f32, kind="ExternalInput").ap()
    outd = nc.dram_tensor("out", [T, D], bf16, kind="ExternalOutput").ap()

    xT_r = xT.rearrange("(kt p) t -> p kt t", p=P)        # [128, 16, T]
    wcat_r = wcat.rearrange("(kt p) n -> p kt n", p=P)    # [128, 16, 512]
    wp_r = wp.rearrange("(ct p) o -> p ct o", p=P)        # [128, 2, D]
    cos_r = cosd.rearrange("(t p) c -> p t c", p=P)       # [128, 16, 64]
    sin_r = sind.rearrange("(t p) c -> p t c", p=P)

    import contextlib as _ctxlib
    with tile.TileContext(nc) as tc, _ctxlib.ExitStack() as _es:
        pc = _es.enter_context(tc.tile_pool(name="const", bufs=1))
        pb = _es.enter_context(tc.tile_pool(name="batch", bufs=1))
        px = _es.enter_context(tc.tile_pool(name="xs", bufs=2))
        pw = _es.enter_context(tc.tile_pool(name="work", bufs=2))
        pat = _es.enter_context(tc.tile_pool(name="attn", bufs=2))
        psm = _es.enter_context(tc.tile_pool(name="small", bufs=4))
        po = _es.enter_context(tc.tile_pool(name="outp", bufs=3))
        prl = _es.enter_context(tc.tile_pool(name="rlp", bufs=2))
        prq = _es.enter_context(tc.tile_pool(name="rlq", bufs=2))
        pyt = _es.enter_context(tc.tile_pool(name="ytp", bufs=2))
        # PSUM budget (8 banks): ppA serves tags pmm (qkv accum + out-proj,
        # phase-disjoint), sc, ya at 2 bufs each = 6 banks; ppB serves ptt
        # (bf16 transpose staging) + lb ([1,512] row) at 1 buf each = 2.
        ppA = _es.enter_context(tc.tile_pool(name="psA", bufs=2, space="PSUM"))
        ppB = _es.enter_context(tc.tile_pool(name="psB", bufs=1, space="PSUM"))

        # ---- constants resident in SBUF
        wcat_sb = pc.tile([P, TT_B, QKV], bf16, tag="wcat")
        for kt in range(TT_B):
            nc.sync.dma_start(wcat_sb[:, kt, :], wcat_r[:, kt, :])
        wp_sb = pc.tile([P, 2, D], bf16, tag="wp")
        nc.sync.dma_start(wp_sb[:], wp_r[:])
        cos_sb = pc.tile([P, TT_B, HD // 2], bf16, tag="cos")
        nc.sync.dma_start(cos_sb[:], cos_r[:])
        sin_sb = pc.tile([P, TT_B, HD // 2], bf16, tag="sin")
        nc.sync.dma_start(sin_sb[:], sin_r[:])
        rmask_sb = pc.tile([P, P], f32, tag="rmask")
        nc.sync.dma_start(rmask_sb[:], rmaskd[:])
        gain_sb = pc.tile([P, 4], f32, tag="gain")
        nc.sync.dma_start(gain_sb[:], gaind[:])
        idf = pc.tile([P, P], f32, tag="idf")
        make_identity(nc, idf[:])
        idb = pc.tile([P, P], bf16, tag="idb")
        nc.vector.tensor_copy(idb[:], idf[:])
        ones_sb = pc.tile([P, P], bf16, tag="ones")
        nc.vector.memset(ones_sb[:], 1.0)

        prev_tail = None
        for b in [bb % B for bb in range(B * max(1, loop_n))]:
            # qkT packs [q0, q1, k] transposed heads: [128, 3, 1024] per half
            qkT_h = [pb.tile([P, 3, S // 2], bf16, tag="qkTlo", name="qkTlo"),
                     pb.tile([P, 3, S // 2], bf16, tag="qkThi", name="qkThi")]
            vN_h = [pb.tile([P, TT_B // 2, HD], bf16, tag="vNlo", name="vNlo"),
                    pb.tile([P, TT_B // 2, HD], bf16, tag="vNhi", name="vNhi")]

            def qT_grp(hh, g):
                half_i, loc = divmod(g * 4, TT_B // 2)
                return qkT_h[half_i][:, hh, loc * P:(loc + 4) * P]

            def kT_at(jt):
                half_i, loc = divmod(jt, TT_B // 2)
                return qkT_h[half_i][:, 2, loc * P:(loc + 1) * P]

            def vN_at(jt):
                half_i, loc = divmod(jt, TT_B // 2)
                return vN_h[half_i][:, loc, :]

            yT = pyt.tile([P, 2, S], bf16, tag="yT")

            # ======= QKV projection + RMS + RoPE (software-pipelined) =====
            # All projection matmuls are emitted first so the PE queue never
            # waits on the DVE/Pool rope chain; rope+transpose quarters are
            # interleaved between attention groups below.
            stgs = {}
            xts = {}
            ssqs = {}
            rsgs = {}

            def project_quarter(qq):
                hh = qq // 2
                if qq % 2 == 0:
                    ssqs[hh] = pb.tile([P, HB, 3], f32, tag=f"ssq{hh}",
                                       name=f"ssq{hh}")
                for tt in range(4 * qq, 4 * qq + 4):
                    t0 = b * S + tt * P
                    if tt % 4 == 0:
                        xt = px.tile([P, TT_B, 4 * P], bf16, tag="xt")
                        xts[qq] = xt
                        nc.sync.dma_start(xt[:], xT_r[:, :, t0:t0 + 4 * P])
                    xt = xts[qq]
                    xoff = (tt % 4) * P

                    pp = ppA.tile([P, QKV], f32, tag="pmm", name="pmm")
                    for kt in range(TT_B):
                        nc.tensor.matmul(pp[:], xt[:, kt, xoff:xoff + P],
                                         wcat_sb[:, kt, :],
                                         start=(kt == 0), stop=(kt == TT_B - 1))

                    # v: rounding copy straight out of PSUM
                    nc.scalar.copy(vN_at(tt), pp[:, 3 * HD:4 * HD])
                    # stage q0,q1,k in SBUF; sum-of-squares per segment
                    stg = pb.tile([P, 3 * HD], f32, tag=f"stg{tt}")
                    stgs[tt] = stg
                    nc.scalar.copy(stg[:], pp[:, :3 * HD])
                    scr = prq.tile([P, 3 * HD], f32, tag="scr")
                    nc.vector.tensor_tensor(scr[:], stg[:], stg[:], AL.mult)
                    nc.vector.tensor_reduce(
                        ssqs[hh][:, tt % HB, :],
                        scr[:].rearrange("p (s x) -> p s x", s=3),
                        axis=AX.X, op=AL.add)

            def rsqrt_half(hh):
                # rs = exp(-0.5*ln(ssq/HD+eps)) * gain
                lnb = pb.tile([P, HB, 3], f32, tag=f"lnb{hh}")
                nc.scalar.activation(lnb[:], ssqs[hh][:], AF.Ln,
                                     scale=1.0 / HD, bias=gain_sb[:, 3:4])
                rsb = pb.tile([P, HB, 3], f32, tag=f"rsb{hh}")
                nc.scalar.activation(rsb[:], lnb[:], AF.Exp, scale=-0.5)
                rsg = pb.tile([P, HB, 3], f32, tag=f"rsg{hh}")
                nc.vector.tensor_tensor(
                    rsg[:], rsb[:],
                    gain_sb[:, None, :3].to_broadcast([P, HB, 3]), AL.mult)
                rsgs[hh] = rsg

            def rope_quarter(qq):
                rsg = rsgs[qq // 2]
                for tt in range(4 * qq, 4 * qq + 4):
                    ppv = stgs[tt][:].rearrange("p (s x) -> p s x", s=3)
                    qn = pw.tile([P, 3, HD], bf16, tag="qn")
                    nc.vector.tensor_tensor(
                        qn[:], ppv,
                        rsg[:, tt % HB, :, None].to_broadcast([P, 3, HD]),
                        AL.mult)

                    # rope: out1 = a*cos + b2*sin ; out2 = b2*cos - a*sin
                    a = qn[:, :, :HD // 2]
                    b2 = qn[:, :, HD // 2:]
                    rpb = pw.tile([P, 3, HD], bf16, tag="rpb")
                    o1 = rpb[:, :, :HD // 2]
                    o2 = rpb[:, :, HD // 2:]
                    t1 = pw.tile([P, 3, HD // 2], bf16, tag="t1")
                    t2 = pw.tile([P, 3, HD // 2], bf16, tag="t2")
                    cb = cos_sb[:, tt:tt + 1, :].to_broadcast([P, 3, HD // 2])
                    sb_ = sin_sb[:, tt:tt + 1, :].to_broadcast([P, 3, HD // 2])
                    nc.gpsimd.tensor_tensor(t1[:], a, cb, AL.mult)
                    nc.vector.tensor_tensor(t2[:], b2, sb_, AL.mult)
                    nc.gpsimd.tensor_tensor(o1, t1[:], t2[:], AL.add)
                    nc.gpsimd.tensor_tensor(t1[:], b2, cb, AL.mult)
                    nc.vector.tensor_tensor(t2[:], a, sb_, AL.mult)
                    nc.vector.tensor_tensor(o2, t1[:], t2[:], AL.subtract)

                    # transpose q0,q1,k into [head_dim, token] (bf16, 1 cyc/row)
                    rpf = rpb[:].rearrange("p s x -> p (s x)")
                    ptq = ppB.tile([P, 3 * P], bf16, tag="ptt", name="ptt")
                    for sseg in range(3):
                        nc.tensor.transpose(ptq[:, sseg * P:(sseg + 1) * P],
                                            rpf[:, sseg * P:(sseg + 1) * P],
                                            idb[:])
                    half_i, loc = divmod(tt, TT_B // 2)
                    nc.scalar.copy(
                        qkT_h[half_i][:, :, loc * P:(loc + 1) * P],
                        ptq[:].rearrange("p (s x) -> p s x", s=3))

            # ================= attention (+ interleaved out-proj) =========
            def proj_block(yT_t, bb, tt_list):
                for tt in tt_list:
                    ob = po.tile([P, D], bf16, tag="ob")
                    for oc in range(4):
                        pout = ppA.tile([P, 512], f32, tag="pmm", name="pout")
                        for ct in range(2):
                            nc.tensor.matmul(
                                pout[:], yT_t[:, ct, tt * P:(tt + 1) * P],
                                wp_sb[:, ct, oc * 512:(oc + 1) * 512],
                                start=(ct == 0), stop=(ct == 1))
                        if oc % 2 == 0:
                            nc.vector.tensor_copy(ob[:, oc * 512:(oc + 1) * 512],
                                                  pout[:])
                        else:
                            nc.scalar.copy(ob[:, oc * 512:(oc + 1) * 512],
                                           pout[:])
                    nc.sync.dma_start(
                        outd[bb * S + tt * P: bb * S + (tt + 1) * P, :], ob[:])

            def attn_group(g):
                nj = 4 * (g + 1)       # key tiles for this group
                for h in range(2):
                    attnT = pat.tile([P, TT_B, 512], bf16, tag="attnT")
                    # l replicated across all 128 partitions (ones stationary
                    # is [128,128]): same matmul cost, and the reciprocal
                    # runs full-width instead of on one partition (3.3us ->
                    # 0.4us on HW), with no partition_broadcast needed.
                    lb = ppB.tile([P, 512], f32, tag="lb", name="lb")
                    ya = ppA.tile([P, 512], f32, tag="ya", name="ya")
                    q_rhs = qT_grp(h, g)

                    def lav(jt, stop):
                        nc.tensor.matmul(lb[:], ones_sb[:],
                                         attnT[:, jt, :],
                                         start=(jt == 0), stop=stop)
                        nc.tensor.matmul(ya[:], vN_at(jt),
                                         attnT[:, jt, :],
                                         start=(jt == 0), stop=stop)

                    prev = None
                    for jt in range(nj):
                        sc = ppA.tile([P, 512], f32, tag="sc", name="sc")
                        jj = jt - 4 * g
                        c0 = jj * 128 if jj > 0 else 0
                        nc.tensor.matmul(sc[:], kT_at(jt), q_rhs,
                                         start=True, stop=True)
                        if jj >= 0:
                            # triangular mask on the boundary block
                            nc.vector.tensor_tensor(
                                sc[:, jj * 128:(jj + 1) * 128],
                                sc[:, jj * 128:(jj + 1) * 128],
                                rmask_sb[:], AL.add)
                        if c0 > 0:
                            nc.vector.memset(attnT[:, jt, :c0], 0.0)
                        nc.scalar.activation(attnT[:, jt, c0:], sc[:, c0:],
                                             AF.Exp)
                        if prev is not None:
                            lav(prev, stop=False)
                        prev = jt
                    lav(prev, stop=True)

                    rlb = prl.tile([P, 512], f32, tag="rlb")
                    nc.vector.reciprocal(rlb[:], lb[:])
                    nc.vector.tensor_tensor(
                        yT[:, h, g * 512:(g + 1) * 512], ya[:], rlb[:],
                        AL.mult)

                    # out-proj for the previous group's tokens: emitted here
                    # so its PE work lands after this group's first head and
                    # never waits on a fresh evac chain.
                    if h == 0 and g >= 1:
                        proj_block(yT, b, range(4 * (g - 1), 4 * g))

            project_quarter(0)
            project_quarter(1)
            rsqrt_half(0)
            project_quarter(2)
            rope_quarter(0)
            project_quarter(3)
            rope_quarter(1)
            rsqrt_half(1)
            attn_group(0)
            rope_quarter(2)
            attn_group(1)
            rope_quarter(3)
            attn_group(2)
            attn_group(3)
            proj_block(yT, b, range(4 * (GROUPS - 1), 4 * GROUPS))

    nc.compile()
    return nc


def _get_program(loop_n=0):
    key = loop_n
    if key not in _PROG:
        _PROG[key] = _build_program(loop_n)
    return _PROG[key]


def _host_prep(x, Wq, Wk, Wv, Wp, q_gain):
    """Build the 8 per-core input maps."""
    import ml_dtypes
    bf16 = ml_dtypes.bfloat16

    x = np.ascontiguousarray(x.reshape(T, D), dtype=np.float32)
    xT = np.ascontiguousarray(x.T.astype(bf16))          # [D, T] bf16

    inv_freq = 1.0 / (ROPE_BASE ** (np.arange(0, HD, 2, dtype=np.float32) / HD))
    freqs = np.arange(S, dtype=np.float32)[:, None] * inv_freq[None, :]
    cos = np.ascontiguousarray(np.cos(freqs).astype(bf16))   # [S, 64]
    sin = np.ascontiguousarray(np.sin(freqs).astype(bf16))

    r = np.arange(P)[:, None]
    c = np.arange(P)[None, :]
    rmask = np.where(c < r, NEG, 0.0).astype(np.float32)   # [128, 128] tri

    in_maps = []
    for core in range(N_CORES):
        h0 = 2 * core
        kv = core // 2
        WqT = Wq[h0 * HD:(h0 + 2) * HD, :].T             # [D, 256]
        WkT = Wk[kv * HD:(kv + 1) * HD, :].T             # [D, 128]
        WvT = Wv[kv * HD:(kv + 1) * HD, :].T             # [D, 128]
        wcat = np.ascontiguousarray(
            np.concatenate([WqT, WkT, WvT], axis=1).astype(bf16))
        wpT = np.ascontiguousarray(
            Wp[:, h0 * HD:(h0 + 2) * HD].T.astype(bf16))        # [256, D]
        scale = 1.0 / math.sqrt(HD)
        gain = np.tile(np.array(
            [[q_gain[h0] * scale, q_gain[h0 + 1] * scale, 1.0, EPS]],
            dtype=np.float32), (P, 1))
        in_maps.append({
            "xT": xT,
            "wcat": wcat,
            "wp": wpT,
            "cosd": cos,
            "sind": sin,
            "rmaskd": rmask,
            "gaind": np.ascontiguousarray(gain),
        })
    return in_maps


def kernel(x, Wq, Wk, Wv, Wp, q_gain):
    from concourse.bass_utils import run_bass_kernel_spmd

    nc = _get_program()
    in_maps = _host_prep(x, Wq, Wk, Wv, Wp, q_gain)
    try:
        res = run_bass_kernel_spmd(nc, in_maps, core_ids=list(range(N_CORES)))
    except Exception:
        # one retry: a previous crashed run can leave the exec unit wedged
        res = run_bass_kernel_spmd(nc, in_maps, core_ids=list(range(N_CORES)))
    total = np.zeros((T, D), dtype=np.float32)
    for r in res.results:
        total += r["out"].astype(np.float32)
    return total.reshape(B, S, D)


# revision 41
# speedup vs baseline: 1.7752x; 1.4296x over previous
"""Causal GQA self-attention (B=2, S=2048, D=2048, 16 heads / 4 KV heads) on 8
Trainium2 NeuronCores.

Sharding: tensor-parallel over heads. Core c owns Q heads (2c, 2c+1) and KV
head c//2. Each core computes its heads' attention output and a partial
output projection (columns of Wp.T owned by its heads); the host sums the 8
partial outputs.

v2 design (vs baseline):
  - All matmuls in bf16 (PSUM accumulation stays f32). x, weights, and the
    partial outputs move over DMA in bf16, halving HBM traffic.
  - Attention scores are computed directly transposed: sc[key, query] =
    kT_tile^T @ qT_group, so the exp'd tiles feed the AV matmul with no PE
    transposes and no PSUM->SBUF copy pass. The softmax denominator comes
    from a ones-stationary matmul accumulated in PSUM [1, 512].
  - Causal masking: only the 4 in-group (diagonal) tiles per (g,h) need a
    triangular mask add; above-diagonal regions exp() to exactly 0 and
    contribute nothing to AV / the denominator.
  - cos/sin RoPE tables are SBUF-resident constants (identical across
    batches), not per-tile DMAs.
"""

import math

import numpy as np

B = 2
S = 2048
D = 2048
T = B * S
NH = 16
NKV = 4
HD = 128
P = 128
ROPE_BASE = 10000.0
EPS = float(np.finfo(np.float32).eps)
NEG = -1.0e30

N_CORES = 8
TT_B = S // P          # 16 token tiles per batch
GROUPS = 4             # groups of 4 q-tiles (512 queries)
QKV = 512              # per-core fused projection width: 2*q + k + v heads
HB = 8                 # token tiles per rsqrt batch

_PROG = {}


def _build_program(loop_n=0):
    import concourse.mybir as mybir
    import concourse.tile as tile
    from concourse import bacc
    from concourse.masks import make_identity

    f32 = mybir.dt.float32
    bf16 = mybir.dt.bfloat16
    AL = mybir.AluOpType
    AF = mybir.ActivationFunctionType
    AX = mybir.AxisListType

    nc = bacc.Bacc("TRN2", target_bir_lowering=False, debug=False,
                   enable_asserts=True, num_devices=N_CORES)

    xT = nc.dram_tensor("xT", [D, T], bf16, kind="ExternalInput").ap()
    wcat = nc.dram_tensor("wcat", [D, QKV], bf16, kind="ExternalInput").ap()
    wp = nc.dram_tensor("wp", [2 * HD, D], bf16, kind="ExternalInput").ap()
    cosd = nc.dram_tensor("cosd", [S, HD // 2], bf16, kind="ExternalInput").ap()
    sind = nc.dram_tensor("sind", [S, HD // 2], bf16, kind="ExternalInput").ap()
    rmaskd = nc.dram_tensor("rmaskd", [P, P], f32, kind="ExternalInput").ap()
    gaind = nc.dram_tensor("gaind", [P, 4], f32, kind="ExternalInput").ap()
    outd = nc.dram_tensor("out", [T, D], bf16, kind="ExternalOutput").ap()

    xT_r = xT.rearrange("(kt p) t -> p kt t", p=P)        # [128, 16, T]
    wcat_r = wcat.rearrange("(kt p) n -> p kt n", p=P)    # [128, 16, 512]
    wp_r = wp.rearrange("(ct p) o -> p ct o", p=P)        # [128, 2, D]
    cos_r = cosd.rearrange("(t p) c -> p t c", p=P)       # [128, 16, 64]
    sin_r = sind.rearrange("(t p) c -> p t c", p=P)

    import contextlib as _ctxlib
    with tile.TileContext(nc) as tc, _ctxlib.ExitStack() as _es:
        pc = _es.enter_context(tc.tile_pool(name="const", bufs=1))
        pb = _es.enter_context(tc.tile_pool(name="batch", bufs=1))
        px = _es.enter_context(tc.tile_pool(name="xs", bufs=2))
        pw = _es.enter_context(tc.tile_pool(name="work", bufs=2))
        pat = _es.enter_context(tc.tile_pool(name="attn", bufs=2))
        psm = _es.enter_context(tc.tile_pool(name="small", bufs=4))
        po = _es.enter_context(tc.tile_pool(name="outp", bufs=3))
        prl = _es.enter_context(tc.tile_pool(name="rlp", bufs=2))
        prq = _es.enter_context(tc.tile_pool(name="rlq", bufs=2))
        pyt = _es.enter_context(tc.tile_pool(name="ytp", bufs=2))
        # PSUM budget (8 banks): ppA serves tags pmm (qkv accum + out-proj,
        # phase-disjoint), sc, ya at 2 bufs each = 6 banks; ppB serves ptt
        # (bf16 transpose staging) + lb ([1,512] row) at 1 buf each = 2.
        ppA = _es.enter_context(tc.tile_pool(name="psA", bufs=2, space="PSUM"))
        ppB = _es.enter_context(tc.tile_pool(name="psB", bufs=1, space="PSUM"))

        # ---- constants resident in SBUF
        wcat_sb = pc.tile([P, TT_B, QKV], bf16, tag="wcat")
        for kt in range(TT_B):
            nc.sync.dma_start(wcat_sb[:, kt, :], wcat_r[:, kt, :])
        wp_sb = pc.tile([P, 2, D], bf16, tag="wp")
        nc.sync.dma_start(wp_sb[:], wp_r[:])
        cos_sb = pc.tile([P, TT_B, HD // 2], bf16, tag="cos")
        nc.sync.dma_start(cos_sb[:], cos_r[:])
        sin_sb = pc.tile([P, TT_B, HD // 2], bf16, tag="sin")
        nc.sync.dma_start(sin_sb[:], sin_r[:])
        rmask_sb = pc.tile([P, P], f32, tag="rmask")
        nc.sync.dma_start(rmask_sb[:], rmaskd[:])
        gain_sb = pc.tile([P, 4], f32, tag="gain")
        nc.sync.dma_start(gain_sb[:], gaind[:])
        idf = pc.tile([P, P], f32, tag="idf")
        make_identity(nc, idf[:])
        idb = pc.tile([P, P], bf16, tag="idb")
        nc.vector.tensor_copy(idb[:], idf[:])
        ones_sb = pc.tile([P, P], bf16, tag="ones")
        nc.vector.memset(ones_sb[:], 1.0)

        prev_tail = None
        for b in [bb % B for bb in range(B * max(1, loop_n))]:
            # qkT packs [q0, q1, k] transposed heads: [128, 3, 1024] per half
            qkT_h = [pb.tile([P, 3, S // 2], bf16, tag="qkTlo", name="qkTlo"),
                     pb.tile([P, 3, S // 2], bf16, tag="qkThi", name="qkThi")]
            vN_h = [pb.tile([P, TT_B // 2, HD], bf16, tag="vNlo", name="vNlo"),
                    pb.tile([P, TT_B // 2, HD], bf16, tag="vNhi", name="vNhi")]

            def qT_grp(hh, g):
                half_i, loc = divmod(g * 4, TT_B // 2)
                return qkT_h[half_i][:, hh, loc * P:(loc + 4) * P]

            def kT_at(jt):
                half_i, loc = divmod(jt, TT_B // 2)
                return qkT_h[half_i][:, 2, loc * P:(loc + 1) * P]

            def vN_at(jt):
                half_i, loc = divmod(jt, TT_B // 2)
                return vN_h[half_i][:, loc, :]

            yT = pyt.tile([P, 2, S], bf16, tag="yT")

            # ======= QKV projection + RMS + RoPE (software-pipelined) =====
            # All projection matmuls are emitted first so the PE queue never
            # waits on the DVE/Pool rope chain; rope+transpose quarters are
            # interleaved between attention groups below.
            stgs = {}
            xts = {}
            ssqs = {}
            rsgs = {}

            def project_quarter(qq):
                hh = qq // 2
                if qq % 2 == 0:
                    ssqs[hh] = pb.tile([P, HB, 3], f32, tag=f"ssq{hh}",
                                       name=f"ssq{hh}")
                for tt in range(4 * qq, 4 * qq + 4):
                    t0 = b * S + tt * P
                    if tt % 4 == 0:
                        xt = px.tile([P, TT_B, 4 * P], bf16, tag="xt")
                        xts[qq] = xt
                        nc.sync.dma_start(xt[:], xT_r[:, :, t0:t0 + 4 * P])
                    xt = xts[qq]
                    xoff = (tt % 4) * P

                    pp = ppA.tile([P, QKV], f32, tag="pmm", name="pmm")
                    for kt in range(TT_B):
                        nc.tensor.matmul(pp[:], xt[:, kt, xoff:xoff + P],
                                         wcat_sb[:, kt, :],
                                         start=(kt == 0), stop=(kt == TT_B - 1))

                    # v: rounding copy straight out of PSUM
                    nc.scalar.copy(vN_at(tt), pp[:, 3 * HD:4 * HD])
                    # stage q0,q1,k in SBUF; sum-of-squares per segment
                    stg = pb.tile([P, 3 * HD], f32, tag=f"stg{tt}")
                    stgs[tt] = stg
                    nc.scalar.copy(stg[:], pp[:, :3 * HD])
                    scr = prq.tile([P, 3 * HD], f32, tag="scr")
                    nc.vector.tensor_tensor(scr[:], stg[:], stg[:], AL.mult)
                    nc.vector.tensor_reduce(
                        ssqs[hh][:, tt % HB, :],
                        scr[:].rearrange("p (s x) -> p s x", s=3),
                        axis=AX.X, op=AL.add)

            def rsqrt_half(hh):
                # rs = exp(-0.5*ln(ssq/HD+eps)) * gain
                lnb = pb.tile([P, HB, 3], f32, tag=f"lnb{hh}")
                nc.scalar.activation(lnb[:], ssqs[hh][:], AF.Ln,
                                     scale=1.0 / HD, bias=gain_sb[:, 3:4])
                rsb = pb.tile([P, HB, 3], f32, tag=f"rsb{hh}")
                nc.scalar.activation(rsb[:], lnb[:], AF.Exp, scale=-0.5)
                rsg = pb.tile([P, HB, 3], f32, tag=f"rsg{hh}")
                nc.vector.tensor_tensor(
                    rsg[:], rsb[:],
                    gain_sb[:, None, :3].to_broadcast([P, HB, 3]), AL.mult)
                rsgs[hh] = rsg

            def rope_quarter(qq):
                rsg = rsgs[qq // 2]
                for tt in range(4 * qq, 4 * qq + 4):
                    ppv = stgs[tt][:].rearrange("p (s x) -> p s x", s=3)
                    qn = pw.tile([P, 3, HD], bf16, tag="qn")
                    nc.vector.tensor_tensor(
                        qn[:], ppv,
                        rsg[:, tt % HB, :, None].to_broadcast([P, 3, HD]),
                        AL.mult)

                    # rope: out1 = a*cos + b2*sin ; out2 = b2*cos - a*sin
                    a = qn[:, :, :HD // 2]
                    b2 = qn[:, :, HD // 2:]
                    rpb = pw.tile([P, 3, HD], bf16, tag="rpb")
                    o1 = rpb[:, :, :HD // 2]
                    o2 = rpb[:, :, HD // 2:]
                    t1 = pw.tile([P, 3, HD // 2], bf16, tag="t1")
                    t2 = pw.tile([P, 3, HD // 2], bf16, tag="t2")
                    cb = cos_sb[:, tt:tt + 1, :].to_broadcast([P, 3, HD // 2])
                    sb_ = sin_sb[:, tt:tt + 1, :].to_broadcast([P, 3, HD // 2])
                    nc.gpsimd.tensor_tensor(t1[:], a, cb, AL.mult)
                    nc.vector.tensor_tensor(t2[:], b2, sb_, AL.mult)
                    nc.gpsimd.tensor_tensor(o1, t1[:], t2[:], AL.add)
                    nc.gpsimd.tensor_tensor(t1[:], b2, cb, AL.mult)
                    nc.vector.tensor_tensor(t2[:], a, sb_, AL.mult)
                    nc.vector.tensor_tensor(o2, t1[:], t2[:], AL.subtract)

                    # transpose q0,q1,k into [head_dim, token] (bf16, 1 cyc/row)
                    rpf = rpb[:].rearrange("p s x -> p (s x)")
                    ptq = ppB.tile([P, 3 * P], bf16, tag="ptt", name="ptt")
                    for sseg in range(3):
                        nc.tensor.transpose(ptq[:, sseg * P:(sseg + 1) * P],
                                            rpf[:, sseg * P:(sseg + 1) * P],
                                            idb[:])
                    half_i, loc = divmod(tt, TT_B // 2)
                    nc.scalar.copy(
                        qkT_h[half_i][:, :, loc * P:(loc + 1) * P],
                        ptq[:].rearrange("p (s x) -> p s x", s=3))

            # ================= attention (+ interleaved out-proj) =========
            def proj_block(yT_t, bb, tt_list):
                for tt in tt_list:
                    ob = po.tile([P, D], bf16, tag="ob")
                    for oc in range(4):
                        pout = ppA.tile([P, 512], f32, tag="pmm", name="pout")
                        for ct in range(2):
                            nc.tensor.matmul(
                                pout[:], yT_t[:, ct, tt * P:(tt + 1) * P],
                                wp_sb[:, ct, oc * 512:(oc + 1) * 512],
                                start=(ct == 0), stop=(ct == 1))
                        if oc % 2 == 0:
                            nc.vector.tensor_copy(ob[:, oc * 512:(oc + 1) * 512],
                                                  pout[:])
                        else:
                            nc.scalar.copy(ob[:, oc * 512:(oc + 1) * 512],
                                           pout[:])
                    nc.sync.dma_start(
                        outd[bb * S + tt * P: bb * S + (tt + 1) * P, :], ob[:])

            def attn_group(g):
                nj = 4 * (g + 1)       # key tiles for this group
                for h in range(2):
                    attnT = pat.tile([P, TT_B, 512], bf16, tag="attnT")
                    # l replicated across all 128 partitions (ones stationary
                    # is [128,128]): same matmul cost, and the reciprocal
                    # runs full-width instead of on one partition (3.3us ->
                    # 0.4us on HW), with no partition_broadcast needed.
                    lb = ppB.tile([P, 512], f32, tag="lb", name="lb")
                    ya = ppA.tile([P, 512], f32, tag="ya", name="ya")
                    q_rhs = qT_grp(h, g)

                    def lav(jt, stop):
                        nc.tensor.matmul(lb[:], ones_sb[:],
                                         attnT[:, jt, :],
                                         start=(jt == 0), stop=stop)
                        nc.tensor.matmul(ya[:], vN_at(jt),
                                         attnT[:, jt, :],
                                         start=(jt == 0), stop=stop)

                    prev = None
                    for jt in range(nj):
                        sc = ppA.tile([P, 512], f32, tag="sc", name="sc")
                        jj = jt - 4 * g
                        c0 = jj * 128 if jj > 0 else 0
                        nc.tensor.matmul(sc[:], kT_at(jt), q_rhs,
                                         start=True, stop=True)
                        if jj >= 0:
                            # triangular mask on the boundary block
                            nc.vector.tensor_tensor(
                                sc[:, jj * 128:(jj + 1) * 128],
                                sc[:, jj * 128:(jj + 1) * 128],
                                rmask_sb[:], AL.add)
                        if c0 > 0:
                            nc.vector.memset(attnT[:, jt, :c0], 0.0)
                        nc.scalar.activation(attnT[:, jt, c0:], sc[:, c0:],
                                             AF.Exp)
                        if prev is not None:
                            lav(prev, stop=False)
                        prev = jt
                    lav(prev, stop=True)

                    rlb = prl.tile([P, 512], f32, tag="rlb")
                    nc.vector.reciprocal_approx_fast(rlb[:], lb[:])
                    nc.vector.tensor_tensor(
                        yT[:, h, g * 512:(g + 1) * 512], ya[:], rlb[:],
                        AL.mult)

                    # out-proj for the previous group's tokens: emitted here
                    # so its PE work lands after this group's first head and
                    # never waits on a fresh evac chain.
                    if h == 0 and g >= 1:
                        proj_block(yT, b, range(4 * (g - 1), 4 * g))

            project_quarter(0)
            project_quarter(1)
            rsqrt_half(0)
            project_quarter(2)
            rope_quarter(0)
            project_quarter(3)
            rope_quarter(1)
            rsqrt_half(1)
            attn_group(0)
            rope_quarter(2)
            attn_group(1)
            rope_quarter(3)
            attn_group(2)
            attn_group(3)
            proj_block(yT, b, range(4 * (GROUPS - 1), 4 * GROUPS))

    nc.compile()
    return nc


def _get_program(loop_n=0):
    key = loop_n
    if key not in _PROG:
        _PROG[key] = _build_program(loop_n)
    return _PROG[key]


def _host_prep(x, Wq, Wk, Wv, Wp, q_gain):
    """Build the 8 per-core input maps."""
    import ml_dtypes
    bf16 = ml_dtypes.bfloat16

    x = np.ascontiguousarray(x.reshape(T, D), dtype=np.float32)
    xT = np.ascontiguousarray(x.T.astype(bf16))          # [D, T] bf16

    inv_freq = 1.0 / (ROPE_BASE ** (np.arange(0, HD, 2, dtype=np.float32) / HD))
    freqs = np.arange(S, dtype=np.float32)[:, None] * inv_freq[None, :]
    cos = np.ascontiguousarray(np.cos(freqs).astype(bf16))   # [S, 64]
    sin = np.ascontiguousarray(np.sin(freqs).astype(bf16))

    r = np.arange(P)[:, None]
    c = np.arange(P)[None, :]
    rmask = np.where(c < r, NEG, 0.0).astype(np.float32)   # [128, 128] tri

    in_maps = []
    for core in range(N_CORES):
        h0 = 2 * core
        kv = core // 2
        WqT = Wq[h0 * HD:(h0 + 2) * HD, :].T             # [D, 256]
        WkT = Wk[kv * HD:(kv + 1) * HD, :].T             # [D, 128]
        WvT = Wv[kv * HD:(kv + 1) * HD, :].T             # [D, 128]
        wcat = np.ascontiguousarray(
            np.concatenate([WqT, WkT, WvT], axis=1).astype(bf16))
        wpT = np.ascontiguousarray(
            Wp[:, h0 * HD:(h0 + 2) * HD].T.astype(bf16))        # [256, D]
        scale = 1.0 / math.sqrt(HD)
        gain = np.tile(np.array(
            [[q_gain[h0] * scale, q_gain[h0 + 1] * scale, 1.0, EPS]],
            dtype=np.float32), (P, 1))
        in_maps.append({
            "xT": xT,
            "wcat": wcat,
            "wp": wpT,
            "cosd": cos,
            "sind": sin,
            "rmaskd": rmask,
            "gaind": np.ascontiguousarray(gain),
        })
    return in_maps


def kernel(x, Wq, Wk, Wv, Wp, q_gain):
    from concourse.bass_utils import run_bass_kernel_spmd

    nc = _get_program()
    in_maps = _host_prep(x, Wq, Wk, Wv, Wp, q_gain)
    try:
        res = run_bass_kernel_spmd(nc, in_maps, core_ids=list(range(N_CORES)))
    except Exception:
        # one retry: a previous crashed run can leave the exec unit wedged
        res = run_bass_kernel_spmd(nc, in_maps, core_ids=list(range(N_CORES)))
    total = np.zeros((T, D), dtype=np.float32)
    for r in res.results:
        total += r["out"].astype(np.float32)
    return total.reshape(B, S, D)


# revision 45
# speedup vs baseline: 2.1303x; 1.2000x over previous
"""Causal GQA self-attention (B=2, S=2048, D=2048, 16 heads / 4 KV heads) on 8
Trainium2 NeuronCores.

Sharding: (kv-head x batch). Core c owns kv head c//2 and batch c%2: it
projects the 4 GQA query heads of that kv head plus k,v for its batch's 2048
tokens (no duplicated k/v work), runs causal attention for those 4 heads, and
computes the partial output projection for its batch's tokens. The host sums
the 4 kv-group partials per batch.

Device-side design (carried over from the head-TP version):
  - All matmuls bf16 (PSUM f32); x/weights/outputs cross DMA as bf16.
  - Scores are computed transposed: sc[key, query] = kT_tile^T @ qT_group,
    feeding AV directly with no PE transposes. The softmax denominator comes
    from a ones[128,128]-stationary matmul -> lb[128,512] (128 replicated
    rows) -> single full-width reciprocal_approx_fast.
  - QKV projection runs in two passes per token tile (q: 512 wide, kv: 256
    wide) to respect PSUM bank limits; RMS+RoPE is software-pipelined so the
    PE queue never waits on the DVE/Pool rope chain.
"""

import math

import numpy as np

B = 2
S = 2048
D = 2048
T = B * S
NH = 16
NKV = 4
HD = 128
P = 128
ROPE_BASE = 10000.0
EPS = float(np.finfo(np.float32).eps)
NEG = -1.0e30

N_CORES = 8
TT_B = S // P          # 16 token tiles per batch
GROUPS = 4             # groups of 4 q-tiles (512 queries)
QW = 512               # q projection width (4 heads)
KVW = 256              # k+v projection width
NSEG = 5               # rms/rope segments: q0..q3, k
HB = 8                 # token tiles per rsqrt batch

_PROG = {}


def _build_program(loop_n=0):
    import concourse.mybir as mybir
    import concourse.tile as tile
    from concourse import bacc
    from concourse.masks import make_identity

    f32 = mybir.dt.float32
    bf16 = mybir.dt.bfloat16
    AL = mybir.AluOpType
    AF = mybir.ActivationFunctionType
    AX = mybir.AxisListType

    nc = bacc.Bacc("TRN2", target_bir_lowering=False, debug=False,
                   enable_asserts=True, num_devices=N_CORES)

    xT = nc.dram_tensor("xT", [D, S], bf16, kind="ExternalInput").ap()
    wcat = nc.dram_tensor("wcat", [D, QW + KVW], bf16,
                          kind="ExternalInput").ap()
    wp = nc.dram_tensor("wp", [4 * HD, D], bf16, kind="ExternalInput").ap()
    cosd = nc.dram_tensor("cosd", [S, HD // 2], bf16,
                          kind="ExternalInput").ap()
    sind = nc.dram_tensor("sind", [S, HD // 2], bf16,
                          kind="ExternalInput").ap()
    rmaskd = nc.dram_tensor("rmaskd", [P, P], f32, kind="ExternalInput").ap()
    gaind = nc.dram_tensor("gaind", [P, 6], f32, kind="ExternalInput").ap()
    outd = nc.dram_tensor("out", [S, D], bf16, kind="ExternalOutput").ap()

    xT_r = xT.rearrange("(kt p) t -> p kt t", p=P)        # [128, 16, S]
    wcat_r = wcat.rearrange("(kt p) n -> p kt n", p=P)    # [128, 16, 768]
    wp_r = wp.rearrange("(ct p) o -> p ct o", p=P)        # [128, 4, D]
    cos_r = cosd.rearrange("(t p) c -> p t c", p=P)       # [128, 16, 64]
    sin_r = sind.rearrange("(t p) c -> p t c", p=P)

    import contextlib as _ctxlib
    with tile.TileContext(nc) as tc, _ctxlib.ExitStack() as _es:
        pc = _es.enter_context(tc.tile_pool(name="const", bufs=1))
        pb = _es.enter_context(tc.tile_pool(name="batch", bufs=1))
        px = _es.enter_context(tc.tile_pool(name="xs", bufs=2))
        pw = _es.enter_context(tc.tile_pool(name="work", bufs=2))
        pat = _es.enter_context(tc.tile_pool(name="attn", bufs=2))
        po = _es.enter_context(tc.tile_pool(name="outp", bufs=3))
        prl = _es.enter_context(tc.tile_pool(name="rlp", bufs=2))
        prq = _es.enter_context(tc.tile_pool(name="rlq", bufs=2))
        # PSUM (8 banks): ppA = {pmm, sc} x 2 bufs = 4; ppB = {pmv, ptt,
        # ya, lb} x 1 buf = 4.
        ppA = _es.enter_context(tc.tile_pool(name="psA", bufs=2, space="PSUM"))
        ppB = _es.enter_context(tc.tile_pool(name="psB", bufs=1, space="PSUM"))

        # ---- constants resident in SBUF
        wcat_sb = pc.tile([P, TT_B, QW + KVW], bf16, tag="wcat")
        for kt in range(TT_B):
            nc.sync.dma_start(wcat_sb[:, kt, :], wcat_r[:, kt, :])
        wp_sb = pc.tile([P, 4, D], bf16, tag="wp")
        nc.sync.dma_start(wp_sb[:], wp_r[:])
        cos_sb = pc.tile([P, TT_B, HD // 2], bf16, tag="cos")
        nc.sync.dma_start(cos_sb[:], cos_r[:])
        sin_sb = pc.tile([P, TT_B, HD // 2], bf16, tag="sin")
        nc.sync.dma_start(sin_sb[:], sin_r[:])
        rmask_sb = pc.tile([P, P], f32, tag="rmask")
        nc.sync.dma_start(rmask_sb[:], rmaskd[:])
        gain_sb = pc.tile([P, 6], f32, tag="gain")
        nc.sync.dma_start(gain_sb[:], gaind[:])
        idf = pc.tile([P, P], f32, tag="idf")
        make_identity(nc, idf[:])
        idb = pc.tile([P, P], bf16, tag="idb")
        nc.vector.tensor_copy(idb[:], idf[:])
        ones_sb = pc.tile([P, P], bf16, tag="ones")
        nc.vector.memset(ones_sb[:], 1.0)

        for _ in range(max(1, loop_n)):
            # qkT packs [q0..q3, k] transposed: [128, 5, 1024] per half
            qkT_h = [pb.tile([P, NSEG, S // 2], bf16, tag="qkTlo",
                             name="qkTlo"),
                     pb.tile([P, NSEG, S // 2], bf16, tag="qkThi",
                             name="qkThi")]
            vN_h = [pb.tile([P, TT_B // 2, HD], bf16, tag="vNlo", name="vNlo"),
                    pb.tile([P, TT_B // 2, HD], bf16, tag="vNhi", name="vNhi")]

            def qT_grp(hh, g):
                half_i, loc = divmod(g * 4, TT_B // 2)
                return qkT_h[half_i][:, hh, loc * P:(loc + 4) * P]

            def kT_at(jt):
                half_i, loc = divmod(jt, TT_B // 2)
                return qkT_h[half_i][:, 4, loc * P:(loc + 1) * P]

            def vN_at(jt):
                half_i, loc = divmod(jt, TT_B // 2)
                return vN_h[half_i][:, loc, :]

            yT = pb.tile([P, 4, S], bf16, tag="yT")

            # ======= QKV projection + RMS + RoPE (software-pipelined) =====
            stgs = {}
            xts = {}
            ssqs = {}
            rsgs = {}

            def project_quarter(qq):
                hh = qq // 2
                if qq % 2 == 0:
                    ssqs[hh] = pb.tile([P, HB, NSEG], f32, tag=f"ssq{hh}",
                                       name=f"ssq{hh}")
                for tt in range(4 * qq, 4 * qq + 4):
                    t0 = tt * P
                    if tt % 4 == 0:
                        xt = px.tile([P, TT_B, 4 * P], bf16, tag="xt")
                        xts[qq] = xt
                        nc.sync.dma_start(xt[:], xT_r[:, :, t0:t0 + 4 * P])
                    xt = xts[qq]
                    xoff = (tt % 4) * P

                    pp = ppA.tile([P, QW], f32, tag="pmm", name="pmm")
                    for kt in range(TT_B):
                        nc.tensor.matmul(pp[:], xt[:, kt, xoff:xoff + P],
                                         wcat_sb[:, kt, :QW],
                                         start=(kt == 0), stop=(kt == TT_B - 1))
                    pv = ppB.tile([P, KVW], f32, tag="pmv", name="pmv")
                    for kt in range(TT_B):
                        nc.tensor.matmul(pv[:], xt[:, kt, xoff:xoff + P],
                                         wcat_sb[:, kt, QW:],
                                         start=(kt == 0), stop=(kt == TT_B - 1))

                    # v: rounding copy straight out of PSUM
                    nc.scalar.copy(vN_at(tt), pv[:, HD:2 * HD])
                    # stage q0..q3,k in SBUF (bf16); sum-of-squares per seg
                    stg = pb.tile([P, NSEG * HD], bf16, tag=f"stg{tt}")
                    stgs[tt] = stg
                    nc.scalar.copy(stg[:, :QW], pp[:])
                    nc.scalar.copy(stg[:, QW:], pv[:, :HD])
                    scr = prq.tile([P, NSEG * HD], f32, tag="scr")
                    nc.vector.tensor_tensor(scr[:], stg[:], stg[:], AL.mult)
                    nc.vector.tensor_reduce(
                        ssqs[hh][:, tt % HB, :],
                        scr[:].rearrange("p (s x) -> p s x", s=NSEG),
                        axis=AX.X, op=AL.add)

            def rsqrt_half(hh):
                # rs = exp(-0.5*ln(ssq/HD+eps)) * gain
                lnb = pb.tile([P, HB, NSEG], f32, tag=f"lnb{hh}")
                nc.scalar.activation(lnb[:], ssqs[hh][:], AF.Ln,
                                     scale=1.0 / HD, bias=gain_sb[:, 5:6])
                rsb = pb.tile([P, HB, NSEG], f32, tag=f"rsb{hh}")
                nc.scalar.activation(rsb[:], lnb[:], AF.Exp, scale=-0.5)
                rsg = pb.tile([P, HB, NSEG], f32, tag=f"rsg{hh}")
                nc.vector.tensor_tensor(
                    rsg[:], rsb[:],
                    gain_sb[:, None, :NSEG].to_broadcast([P, HB, NSEG]),
                    AL.mult)
                rsgs[hh] = rsg

            def rope_quarter(qq):
                rsg = rsgs[qq // 2]
                for tt in range(4 * qq, 4 * qq + 4):
                    ppv = stgs[tt][:].rearrange("p (s x) -> p s x", s=NSEG)
                    qn = pw.tile([P, NSEG, HD], bf16, tag="qn")
                    nc.vector.tensor_tensor(
                        qn[:], ppv,
                        rsg[:, tt % HB, :, None].to_broadcast([P, NSEG, HD]),
                        AL.mult)

                    # rope: out1 = a*cos + b2*sin ; out2 = b2*cos - a*sin
                    a = qn[:, :, :HD // 2]
                    b2 = qn[:, :, HD // 2:]
                    rpb = pw.tile([P, NSEG, HD], bf16, tag="rpb")
                    o1 = rpb[:, :, :HD // 2]
                    o2 = rpb[:, :, HD // 2:]
                    t1 = pw.tile([P, NSEG, HD // 2], bf16, tag="t1")
                    t2 = pw.tile([P, NSEG, HD // 2], bf16, tag="t2")
                    cb = cos_sb[:, tt:tt + 1, :].to_broadcast(
                        [P, NSEG, HD // 2])
                    sb_ = sin_sb[:, tt:tt + 1, :].to_broadcast(
                        [P, NSEG, HD // 2])
                    nc.gpsimd.tensor_tensor(t1[:], a, cb, AL.mult)
                    nc.vector.tensor_tensor(t2[:], b2, sb_, AL.mult)
                    nc.gpsimd.tensor_tensor(o1, t1[:], t2[:], AL.add)
                    nc.gpsimd.tensor_tensor(t1[:], b2, cb, AL.mult)
                    nc.vector.tensor_tensor(t2[:], a, sb_, AL.mult)
                    nc.vector.tensor_tensor(o2, t1[:], t2[:], AL.subtract)

                    # transpose q0..q3,k into [head_dim, token] (bf16)
                    rpf = rpb[:].rearrange("p s x -> p (s x)")
                    ptq = ppB.tile([P, NSEG * P], bf16, tag="ptt", name="ptt")
                    for sseg in range(NSEG):
                        nc.tensor.transpose(ptq[:, sseg * P:(sseg + 1) * P],
                                            rpf[:, sseg * P:(sseg + 1) * P],
                                            idb[:])
                    half_i, loc = divmod(tt, TT_B // 2)
                    nc.scalar.copy(
                        qkT_h[half_i][:, :, loc * P:(loc + 1) * P],
                        ptq[:].rearrange("p (s x) -> p s x", s=NSEG))

            # ================= attention (+ interleaved out-proj) =========
            def proj_block(tt_list):
                for tt in tt_list:
                    ob = po.tile([P, D], bf16, tag="ob")
                    for oc in range(4):
                        pout = ppA.tile([P, 512], f32, tag="pmm", name="pout")
                        for ct in range(4):
                            nc.tensor.matmul(
                                pout[:], yT[:, ct, tt * P:(tt + 1) * P],
                                wp_sb[:, ct, oc * 512:(oc + 1) * 512],
                                start=(ct == 0), stop=(ct == 3))
                        if oc % 2 == 0:
                            nc.vector.tensor_copy(
                                ob[:, oc * 512:(oc + 1) * 512], pout[:])
                        else:
                            nc.scalar.copy(ob[:, oc * 512:(oc + 1) * 512],
                                           pout[:])
                    nc.sync.dma_start(
                        outd[tt * P:(tt + 1) * P, :], ob[:])

            def attn_group(g):
                nj = 4 * (g + 1)       # key tiles for this group
                for h in range(4):
                    attnT = pat.tile([P, TT_B, 512], bf16, tag="attnT")
                    # l replicated across 128 partitions: same matmul cost,
                    # full-width reciprocal, no partition broadcast.
                    lb = ppB.tile([P, 512], f32, tag="lb", name="lb")
                    ya = ppB.tile([P, 512], f32, tag="ya", name="ya")
                    q_rhs = qT_grp(h, g)

                    def lav(jt, stop):
                        nc.tensor.matmul(lb[:], ones_sb[:],
                                         attnT[:, jt, :],
                                         start=(jt == 0), stop=stop)
                        nc.tensor.matmul(ya[:], vN_at(jt),
                                         attnT[:, jt, :],
                                         start=(jt == 0), stop=stop)

                    prev = None
                    for jt in range(nj):
                        sc = ppA.tile([P, 512], f32, tag="sc", name="sc")
                        jj = jt - 4 * g
                        c0 = jj * 128 if jj > 0 else 0
                        nc.tensor.matmul(sc[:], kT_at(jt), q_rhs,
                                         start=True, stop=True)
                        if jj >= 0:
                            # triangular mask on the boundary block
                            nc.vector.tensor_tensor(
                                sc[:, jj * 128:(jj + 1) * 128],
                                sc[:, jj * 128:(jj + 1) * 128],
                                rmask_sb[:], AL.add)
                        if c0 > 0:
                            nc.vector.memset(attnT[:, jt, :c0], 0.0)
                        nc.scalar.activation(attnT[:, jt, c0:], sc[:, c0:],
                                             AF.Exp)
                        if prev is not None:
                            lav(prev, stop=False)
                        prev = jt
                    lav(prev, stop=True)

                    rlb = prl.tile([P, 512], f32, tag="rlb")
                    nc.vector.reciprocal_approx_fast(rlb[:], lb[:])
                    nc.vector.tensor_tensor(
                        yT[:, h, g * 512:(g + 1) * 512], ya[:], rlb[:],
                        AL.mult)

                    # out-proj for the previous group's tokens, interleaved
                    # one token tile per head so the PE stream stays dense.
                    if g >= 1:
                        proj_block([4 * (g - 1) + h])

            project_quarter(0)
            project_quarter(1)
            rsqrt_half(0)
            project_quarter(2)
            rope_quarter(0)
            project_quarter(3)
            rope_quarter(1)
            rsqrt_half(1)
            attn_group(0)
            rope_quarter(2)
            attn_group(1)
            rope_quarter(3)
            attn_group(2)
            attn_group(3)
            proj_block(range(4 * (GROUPS - 1), 4 * GROUPS))

    nc.compile()
    return nc


def _get_program(loop_n=0):
    key = loop_n
    if key not in _PROG:
        _PROG[key] = _build_program(loop_n)
    return _PROG[key]


def _host_prep(x, Wq, Wk, Wv, Wp, q_gain):
    """Build the 8 per-core input maps: core c -> (kv head c//2, batch c%2)."""
    import ml_dtypes
    bf16 = ml_dtypes.bfloat16

    x = np.ascontiguousarray(x.reshape(B, S, D), dtype=np.float32)
    xTb = [np.ascontiguousarray(x[b].T.astype(bf16)) for b in range(B)]

    inv_freq = 1.0 / (ROPE_BASE ** (np.arange(0, HD, 2, dtype=np.float32) / HD))
    freqs = np.arange(S, dtype=np.float32)[:, None] * inv_freq[None, :]
    cos = np.ascontiguousarray(np.cos(freqs).astype(bf16))   # [S, 64]
    sin = np.ascontiguousarray(np.sin(freqs).astype(bf16))

    r = np.arange(P)[:, None]
    c = np.arange(P)[None, :]
    rmask = np.where(c < r, NEG, 0.0).astype(np.float32)   # [128, 128] tri

    scale = 1.0 / math.sqrt(HD)
    in_maps = []
    for core in range(N_CORES):
        kv = core // 2
        b = core % 2
        h0 = 4 * kv
        WqT = Wq[h0 * HD:(h0 + 4) * HD, :].T             # [D, 512]
        WkT = Wk[kv * HD:(kv + 1) * HD, :].T             # [D, 128]
        WvT = Wv[kv * HD:(kv + 1) * HD, :].T             # [D, 128]
        wcat = np.ascontiguousarray(
            np.concatenate([WqT, WkT, WvT], axis=1).astype(bf16))
        wpT = np.ascontiguousarray(
            Wp[:, h0 * HD:(h0 + 4) * HD].T.astype(bf16))        # [512, D]
        gain = np.tile(np.array(
            [[q_gain[h0] * scale, q_gain[h0 + 1] * scale,
              q_gain[h0 + 2] * scale, q_gain[h0 + 3] * scale, 1.0, EPS]],
            dtype=np.float32), (P, 1))
        in_maps.append({
            "xT": xTb[b],
            "wcat": wcat,
            "wp": wpT,
            "cosd": cos,
            "sind": sin,
            "rmaskd": rmask,
            "gaind": np.ascontiguousarray(gain),
        })
    return in_maps


def kernel(x, Wq, Wk, Wv, Wp, q_gain):
    from concourse.bass_utils import run_bass_kernel_spmd

    nc = _get_program()
    in_maps = _host_prep(x, Wq, Wk, Wv, Wp, q_gain)
    try:
        res = run_bass_kernel_spmd(nc, in_maps, core_ids=list(range(N_CORES)))
    except Exception:
        # one retry: a previous crashed run can leave the exec unit wedged
        res = run_bass_kernel_spmd(nc, in_maps, core_ids=list(range(N_CORES)))
    total = np.zeros((B, S, D), dtype=np.float32)
    for core, r in enumerate(res.results):
        total[core % 2] += r["out"].astype(np.float32)
    return total
